# revision 7
# baseline (speedup 1.0000x reference)
"""AttnReadout (segment softmax readout) Trainium2 kernel.

Math (reference):
  f = BN(feat) = feat*A + B        A = gamma*rsqrt(var+eps), B = beta-mean*A
  e = sigmoid(f@W_u.T + (f[last]@W_i.T + b_i)[seg]) @ w_e
  alpha = segment_softmax(e)
  rst = segsum(f*alpha);  position_rst = segsum(f*pw)

Device strategy (8 cores, node-dim sharded, 131072 nodes = 2048 segs/core):
  pass1 (native feat):  colsum, sumsq (PE ones-matmuls, PSUM accum),
                        P = segsum(pw*feat), spw = segsum(pw)  (PE pwblk matmuls)
  allreduce colsum/sumsq -> A,B on device -> fold into weights:
                        WuaT = A*W_u.T, WiaT = A*W_i.T, c = B@W_u.T+B@W_i.T+b_i
  pass2 (host-transposed featT, j-major cols: c = 128*j + g per supertile):
      zT = WuaT.T@featT + FL[seg]   (PE; FL added via periodic indicator matmul)
      t = tanh(0.5*zT + 0.5*c)      (ACT; sigmoid = .5+.5*tanh, const folds out
                                     of softmax so e' = (0.5*w_e)@t)
      e'-pack: per 128-node j-block, matmul(lhsT=th_block, rhs=we) -> one PSUM
               column => e' lands packed [128seg x 64j]; exp on packed (cheap)
      ex broadcast: PE-transpose -> row [1,8192] -> gpsimd partition_broadcast
      wft = ft*ex_rep (DVE 2x); segment sums via 6-level pairwise add tree
      (j-major makes tree levels contiguous)  -> sexT [f, g]
  host: rst = A*(sex/denom)+B ; position_rst = A*P + B*spw
"""

import numpy as np

N_NODES = 1048576
N_SEG = 16384
SEG = 64
D = 128
EPS = 1e-5
NCORES = 8

_CACHE = {}
LAST_RESULT = None  # BassKernelResults of the most recent kernel() call


def _build_program(n_cores, S):
    """Build + compile the per-core program. S = nodes per shard."""
    import concourse.bass as bass
    import concourse.tile as tile
    from concourse import bacc, mybir

    NSEGS = S // SEG          # segments per shard
    NSUP = S // 8192          # supertiles (128 segs each)
    assert S % 8192 == 0

    nc = bacc.Bacc(
        "TRN2",
        target_bir_lowering=False,
        debug=False,
        enable_asserts=False,
        num_devices=n_cores,
    )
    dtf = mybir.dt.float32
    dth = mybir.dt.bfloat16
    F32 = mybir.ActivationFunctionType

    featN = nc.dram_tensor("featN", [S, D], dth, kind="ExternalInput").ap()
    featT = nc.dram_tensor("featT", [D, S], dth, kind="ExternalInput").ap()
    lastT = nc.dram_tensor("lastT", [D, NSEGS], dth, kind="ExternalInput").ap()
    pwm = nc.dram_tensor("pwm", [D, S // D], dth, kind="ExternalInput").ap()
    wut = nc.dram_tensor("wut", [D, D], dtf, kind="ExternalInput").ap()
    wit = nc.dram_tensor("wit", [D, D], dtf, kind="ExternalInput").ap()
    smalls = nc.dram_tensor("smalls", [D, 4], dtf, kind="ExternalInput").ap()
    ind128 = nc.dram_tensor("ind128", [D, 512], dth, kind="ExternalInput").ap()
    ident = nc.dram_tensor("ident", [D, D], dtf, kind="ExternalInput").ap()
    identh = nc.dram_tensor("identh", [D, D], dth, kind="ExternalInput").ap()

    sexT = nc.dram_tensor("sexT", [D, NSEGS], dtf, kind="ExternalOutput").ap()
    den = nc.dram_tensor("den", [D, NSUP], dtf, kind="ExternalOutput").ap()
    poutT = nc.dram_tensor("poutT", [D, NSEGS], dtf, kind="ExternalOutput").ap()
    statsout = nc.dram_tensor("statsout", [D, 2], dtf, kind="ExternalOutput").ap()

    AL = mybir.AluOpType

    with tile.TileContext(nc) as tc:
        from contextlib import ExitStack

        with ExitStack() as ctx:
            singles = ctx.enter_context(tc.tile_pool(name="singles", bufs=1))

            wut_sb = singles.tile([D, D], dtf)
            nc.sync.dma_start(wut_sb[:], wut)
            wit_sb = singles.tile([D, D], dtf)
            nc.sync.dma_start(wit_sb[:], wit)
            smalls_sb = singles.tile([D, 4], dtf)
            nc.sync.dma_start(smalls_sb[:], smalls)
            ind128_sb = singles.tile([D, 512], dth)
            nc.sync.dma_start(ind128_sb[:], ind128)
            ident_sb = singles.tile([D, D], dtf)
            nc.sync.dma_start(ident_sb[:], ident)
            identh_sb = singles.tile([D, D], dth)
            nc.sync.dma_start(identh_sb[:], identh)
            gamma_c = smalls_sb[:, 0:1]
            beta_c = smalls_sb[:, 1:2]
            bi_c = smalls_sb[:, 2:3]
            we_c = smalls_sb[:, 3:4]
            we_half = singles.tile([D, 1], dtf)
            nc.vector.tensor_scalar_mul(we_half[:], we_c, 0.5)
            we_bf = singles.tile([D, 1], dth)
            nc.vector.tensor_copy(we_bf[:], we_half[:])
            bi_half = singles.tile([D, 1], dtf)
            nc.vector.tensor_scalar_mul(bi_half[:], bi_c, 0.5)

            # psum accumulators for global stats live through pass1+stats
            with tc.tile_pool(name="psacc", bufs=1, space="PSUM") as psacc:
             psum_gram = psacc.tile([D, D], dtf)    # feat.T@feat; diag = sumsq
             cs_accum = singles.tile([D, 1], dtf)
             nc.vector.memset(cs_accum[:], 0.0)

             # ---------------- PASS 1 : native layout ----------------
             with tc.tile_pool(name="p1in", bufs=3) as p1in, \
                  tc.tile_pool(name="p1pw", bufs=2) as p1pw, \
                  tc.tile_pool(name="pblk", bufs=1) as pblk, \
                  tc.tile_pool(name="psP", bufs=2, space="PSUM") as psP:

                 # combo3 per supertile: col 3t = pw upper half of tile t,
                 # 3t+1 = pw lower half, 3t+2 = ones (per-tile colsum lane)
                 combo = []
                 for k in range(2):
                     t = pblk.tile([D, 192], dth, tag=f"combo{k}")
                     nc.vector.memset(t[:], 0.0)
                     ap_ones = t[:, :].rearrange("p (t three) -> p t three",
                                                 three=3)[:, :, 2:3]
                     nc.vector.memset(ap_ones, 1.0)
                     combo.append(t)

                 for s in range(NSUP):
                     cmb = combo[s % 2]
                     pwm_st = p1pw.tile([D, SEG], dth)
                     nc.sync.dma_start(pwm_st[:], pwm[:, SEG * s:SEG * (s + 1)])
                     nc.vector.tensor_copy(
                         cmb[0:SEG, :].rearrange("p (t three) -> p t three",
                                                 three=3)[:, :, 0:1],
                         pwm_st[0:SEG, :],
                     )
                     nc.vector.tensor_copy(
                         cmb[SEG:D, :].rearrange("p (t three) -> p t three",
                                                 three=3)[:, :, 1:2],
                         pwm_st[SEG:D, :],
                     )

                     psum_PT = psP.tile([D, 192], dtf)
                     for b in range(2):  # big tiles: 4096 nodes each
                         base = 8192 * s + 4096 * b
                         ftn = p1in.tile([D, 32, D], dth)
                         nc.sync.dma_start(
                             ftn[:],
                             featN[base:base + 4096, :].rearrange(
                                 "(j p) d -> p j d", p=D
                             ),
                         )
                         first = (s == 0 and b == 0)
                         last = (s == NSUP - 1 and b == 1)
                         for j in range(32):  # 128-node tiles; ftn slice is lhsT
                             t_sup = 32 * b + j
                             lhs = ftn[:, j, :]
                             nc.tensor.matmul(
                                 psum_gram[:], lhs, lhs,
                                 start=(first and j == 0),
                                 stop=(last and j == 31),
                             )
                             mm = nc.tensor.matmul(
                                 psum_PT[:, 3 * t_sup:3 * t_sup + 3],
                                 lhs, cmb[:, 3 * t_sup:3 * t_sup + 3],
                                 start=True, stop=True,
                             )
                             mm.ins.ldweights = False
                     PT_sb = p1pw.tile([D, 192], dtf, tag="PT_sb")
                     nc.vector.tensor_copy(PT_sb[:], psum_PT[:])
                     # compact P pairs (cols 3t,3t+1) then one contiguous DMA
                     P_pack = p1pw.tile([D, D], dtf, tag="P_pack")
                     nc.vector.tensor_copy(
                         P_pack[:].rearrange("p (t two) -> p t two", two=2),
                         PT_sb[:].rearrange("p (t three) -> p t three",
                                            three=3)[:, :, 0:2],
                     )
                     nc.scalar.dma_start(poutT[:, D * s:D * (s + 1)], P_pack[:])
                     # colsum lanes (cols 3t+2) -> accumulate
                     csred = p1pw.tile([D, 1], dtf, tag="csred")
                     nc.vector.tensor_reduce(
                         csred[:],
                         PT_sb[:].rearrange("p (t three) -> p t three",
                                            three=3)[:, :, 2:3],
                         axis=mybir.AxisListType.XY, op=AL.add,
                     )
                     nc.vector.tensor_add(cs_accum[:], cs_accum[:], csred[:])

             # ---------------- STATS: allreduce + fold ----------------
             with tc.tile_pool(name="dram", bufs=1, space="DRAM") as dram, \
                  tc.tile_pool(name="stat", bufs=1) as stat, \
                  tc.tile_pool(name="psstat", bufs=1, space="PSUM") as psstat:
                 stats_col = stat.tile([D, 2], dtf)
                 nc.vector.tensor_copy(stats_col[:, 0:1], cs_accum[:])
                 # sumsq = diag(gram): mask with identity and row-reduce
                 gram_sb = stat.tile([D, D], dtf)
                 nc.vector.tensor_copy(gram_sb[:], psum_gram[:])
                 gmask = stat.tile([D, D], dtf)
                 nc.vector.tensor_mul(gmask[:], gram_sb[:], ident_sb[:])
                 nc.vector.tensor_reduce(
                     stats_col[:, 1:2], gmask[:],
                     axis=mybir.AxisListType.X, op=AL.add,
                 )
                 cc_in = dram.tile([D, 2], dtf)
                 cc_out = dram.tile([D, 2], dtf)
                 nc.sync.dma_start(cc_in[:], stats_col[:])
                 nc.gpsimd.collective_compute(
                     "AllReduce",
                     AL.add,
                     replica_groups=[list(range(n_cores))],
                     ins=[cc_in[:].opt()],
                     outs=[cc_out[:].opt()],
                 )
                 gstats = stat.tile([D, 2], dtf)
                 nc.sync.dma_start(gstats[:], cc_out[:])
                 nc.sync.dma_start(statsout, gstats[:])

                 n_tot = float(n_cores * S)
                 mean_c = stat.tile([D, 1], dtf)
                 nc.vector.tensor_scalar_mul(mean_c[:], gstats[:, 0:1], 1.0 / n_tot)
                 ex2_c = stat.tile([D, 1], dtf)
                 nc.vector.tensor_scalar_mul(ex2_c[:], gstats[:, 1:2], 1.0 / n_tot)
                 m2 = stat.tile([D, 1], dtf)
                 nc.vector.tensor_mul(m2[:], mean_c[:], mean_c[:])
                 var_c = stat.tile([D, 1], dtf)
                 nc.vector.tensor_sub(var_c[:], ex2_c[:], m2[:])
                 eps_t = stat.tile([D, 1], dtf)
                 nc.vector.memset(eps_t[:], EPS)
                 sd_c = stat.tile([D, 1], dtf)
                 nc.scalar.activation(sd_c[:], var_c[:], F32.Sqrt, bias=eps_t[:], scale=1.0)
                 rstd_c = stat.tile([D, 1], dtf)
                 nc.vector.reciprocal(rstd_c[:], sd_c[:])
                 A_c = stat.tile([D, 1], dtf)
                 nc.vector.tensor_mul(A_c[:], rstd_c[:], gamma_c)
                 mA = stat.tile([D, 1], dtf)
                 nc.vector.tensor_mul(mA[:], mean_c[:], A_c[:])
                 B_c = stat.tile([D, 1], dtf)
                 nc.vector.tensor_sub(B_c[:], beta_c, mA[:])

                 wuat = singles.tile([D, D], dth)
                 nc.vector.tensor_scalar_mul(wuat[:], wut_sb[:], A_c[:])
                 wiat = singles.tile([D, D], dth)
                 nc.vector.tensor_scalar_mul(wiat[:], wit_sb[:], A_c[:])
                 wsum = stat.tile([D, D], dtf)
                 nc.vector.tensor_add(wsum[:], wut_sb[:], wit_sb[:])
                 ps_c = psstat.tile([D, 1], dtf)
                 nc.tensor.matmul(ps_c[:], wsum[:], B_c[:], start=True, stop=True)
                 c_half = singles.tile([D, 1], dtf)
                 nc.vector.scalar_tensor_tensor(
                     c_half[:], ps_c[:], 0.5, bi_half[:], AL.mult, AL.add
                 )

            # ---------------- PASS 2 : transposed layout (j-major) ----------
            # fl pre-pass: FL[g,h] for all segs (frees psum banks for main loop)
            fl_all = singles.tile([D, NSUP, D], dth)
            with tc.tile_pool(name="flpre", bufs=2) as flprep, \
                 tc.tile_pool(name="psflp", bufs=2, space="PSUM") as psflp:
                for s in range(NSUP):
                    lt = flprep.tile([D, D], dth)
                    nc.sync.dma_start(lt[:], lastT[:, D * s:D * (s + 1)])
                    psum_fl = psflp.tile([D, D], dtf)
                    nc.tensor.matmul(psum_fl[:], lt[:], wiat[:], start=True, stop=True)
                    nc.vector.tensor_copy(fl_all[:, s, :], psum_fl[:])

            with tc.tile_pool(name="ft", bufs=2) as ftp, \
                 tc.tile_pool(name="th", bufs=3) as thp, \
                 tc.tile_pool(name="expk", bufs=2) as expkp, \
                 tc.tile_pool(name="ext", bufs=2) as extp, \
                 tc.tile_pool(name="row", bufs=2) as rowp, \
                 tc.tile_pool(name="exr", bufs=2) as exrp, \
                 tc.tile_pool(name="wft", bufs=2) as wftp, \
                 tc.tile_pool(name="tre", bufs=2) as trep, \
                 tc.tile_pool(name="sout", bufs=2) as soutp, \
                 tc.tile_pool(name="denall", bufs=1) as denallp, \
                 tc.tile_pool(name="psz", bufs=2, space="PSUM") as psz, \
                 tc.tile_pool(name="pse", bufs=2, space="PSUM") as pse, \
                 tc.tile_pool(name="ptr", bufs=2, space="PSUM") as ptrp:

                den_all = denallp.tile([D, NSUP], dtf)

                for s in range(NSUP):
                    ft = ftp.tile([D, 8192], dth)
                    nc.sync.dma_start(ft[:], featT[:, 8192 * s:8192 * (s + 1)])

                    psum_e = pse.tile([D, SEG], dtf)   # e' packed [g, j]
                    # phase A: scores; 8 psum tiles of 1024 cols (8 j-blocks)
                    for t in range(8):
                        psum_z = psz.tile([D, 1024], dtf)
                        for m in range(2):
                            mm = nc.tensor.matmul(
                                psum_z[:, 512 * m:512 * (m + 1)],
                                wuat[:],
                                ft[:, 1024 * t + 512 * m:1024 * t + 512 * (m + 1)],
                                start=True, stop=False,
                            )
                            if m == 1:
                                mm.ins.ldweights = False
                        for m in range(2):
                            mm = nc.tensor.matmul(
                                psum_z[:, 512 * m:512 * (m + 1)],
                                fl_all[:, s, :],
                                ind128_sb[:],
                                start=False, stop=True,
                            )
                            if m == 1:
                                mm.ins.ldweights = False
                        th_t = thp.tile([D, 1024], dth)
                        nc.scalar.activation(
                            th_t[:], psum_z[:], F32.Tanh, bias=c_half[:], scale=0.5
                        )
                        # e'-pack: one PSUM column per 128-node j-block
                        for b in range(8):
                            jj = 8 * t + b
                            nc.tensor.matmul(
                                psum_e[:, jj:jj + 1],
                                th_t[:, D * b:D * (b + 1)],
                                we_bf[:],
                                start=True, stop=True,
                            )
                    # exp on packed + denominators
                    ex_pk = expkp.tile([D, SEG], dth)
                    nc.scalar.activation(ex_pk[:], psum_e[:], F32.Exp)
                    nc.vector.tensor_reduce(
                        den_all[:, s:s + 1], ex_pk[:],
                        axis=mybir.AxisListType.X, op=AL.add,
                    )
                    # broadcast ex to [f, c]: transpose -> row -> bcast-AP DMA
                    # (scalar HWDGE queue: keeps the sync queue free for the
                    # ft prefetches, which would otherwise block behind these)
                    ptr_t = ptrp.tile([SEG, D], dth)
                    nc.tensor.transpose(ptr_t[:], ex_pk[:], identh_sb[:])
                    exT = extp.tile([SEG, D], dth)
                    nc.vector.tensor_copy(exT[:], ptr_t[:])
                    row = rowp.tile([1, 8192], dth)
                    nc.scalar.dma_start(
                        row[:].rearrange("o (j g) -> o j g", j=SEG), exT[:]
                    )
                    ex_rep = exrp.tile([D, 8192], dth)
                    nc.gpsimd.partition_broadcast(
                        ex_rep[:, 0:4096], row[:, 0:4096]
                    )
                    nc.gpsimd.partition_broadcast(
                        ex_rep[:, 4096:8192], row[:, 4096:8192]
                    )

                    # weighted features + segment-sum tree (j-major: contiguous)
                    wft = wftp.tile([D, 8192], dth)
                    for q in range(8):
                        nc.vector.tensor_mul(
                            wft[:, 1024 * q:1024 * (q + 1)],
                            ft[:, 1024 * q:1024 * (q + 1)],
                            ex_rep[:, 1024 * q:1024 * (q + 1)],
                        )
                    t1 = trep.tile([D, 4096], dth, tag="t1")
                    nc.vector.tensor_add(t1[:], wft[:, 0:4096], wft[:, 4096:8192])
                    t2 = trep.tile([D, 2048], dth, tag="t2")
                    nc.vector.tensor_add(t2[:], t1[:, 0:2048], t1[:, 2048:4096])
                    t3 = trep.tile([D, 1024], dth, tag="t3")
                    nc.vector.tensor_add(t3[:], t2[:, 0:1024], t2[:, 1024:2048])
                    t4 = trep.tile([D, 512], dth, tag="t4")
                    nc.vector.tensor_add(t4[:], t3[:, 0:512], t3[:, 512:1024])
                    t5 = trep.tile([D, 256], dtf, tag="t5")
                    nc.vector.tensor_add(t5[:], t4[:, 0:256], t4[:, 256:512])
                    sexG = soutp.tile([D, D], dtf)
                    nc.vector.tensor_add(sexG[:], t5[:, 0:128], t5[:, 128:256])
                    nc.scalar.dma_start(sexT[:, D * s:D * (s + 1)], sexG[:])

                nc.scalar.dma_start(den, den_all[:])

    nc.compile()
    return nc


def _get_program(n_cores, S):
    key = (n_cores, S)
    if key not in _CACHE:
        _CACHE[key] = _build_program(n_cores, S)
    return _CACHE[key]


def _prep_core_inputs(feat_sh, pw_sh, W_u, W_i, b_i, w_e, gamma, beta):
    S = feat_sh.shape[0]
    NSUP = S // 8192
    import ml_dtypes
    f16 = ml_dtypes.bfloat16
    # j-major transposed layout: col (s, j, g) = node 8192*s + 64*g + j
    featT = np.ascontiguousarray(
        feat_sh.reshape(NSUP, 128, SEG, D).transpose(3, 0, 2, 1).reshape(D, S)
    ).astype(f16)
    lastT = np.ascontiguousarray(feat_sh[SEG - 1::SEG, :].T).astype(f16)
    pwm = np.ascontiguousarray(pw_sh.reshape(-1, D).T).astype(f16)
    # FL periodic indicator: ind128[g, c] = 1 iff c % 128 == g
    ind128 = np.tile(np.eye(D, dtype=np.float32), (1, 4)).astype(f16)
    smalls = np.stack([gamma, beta, b_i, w_e], axis=1).astype(np.float32)
    return {
        "featN": np.ascontiguousarray(feat_sh).astype(f16),
        "featT": featT,
        "lastT": lastT,
        "pwm": pwm,
        "wut": np.ascontiguousarray(W_u.T),
        "wit": np.ascontiguousarray(W_i.T),
        "smalls": smalls,
        "ind128": ind128,
        "ident": np.eye(D, dtype=np.float32),
        "identh": np.eye(D, dtype=np.float32).astype(f16),
    }


def _finalize(results, n_cores, S, gamma, beta, pw):
    NSEGS = S // SEG
    spw_all = pw.astype(np.float64).reshape(-1, SEG).sum(1).astype(np.float32)
    st = results[0]["statsout"]            # [D, 2]
    n_tot = float(n_cores * S)
    mean = st[:, 0] / n_tot
    var = st[:, 1] / n_tot - mean * mean
    A = gamma / np.sqrt(var + EPS)
    B = beta - mean * A
    rst = np.empty((n_cores * NSEGS, D), dtype=np.float32)
    pos = np.empty((n_cores * NSEGS, D), dtype=np.float32)
    for c in range(n_cores):
        r = results[c]
        sex = r["sexT"].T                      # [NSEGS, D]
        denom = r["den"].T.reshape(-1)         # seg order
        p = r["poutT"].T                       # [NSEGS, D]
        sl = slice(c * NSEGS, (c + 1) * NSEGS)
        spw = spw_all[sl]
        rst[sl] = A * (sex / denom[:, None]) + B
        pos[sl] = A * p + B * spw[:, None]
    return rst, pos


def kernel(feat, position_weight, last_nodes, segment_ids, gamma, beta,
           W_u, W_i, b_i, w_e, num_segments):
    from concourse.bass_utils import run_bass_kernel_spmd

    feat = np.asarray(feat, dtype=np.float32)
    pw = np.asarray(position_weight, dtype=np.float32)
    gamma = np.asarray(gamma, dtype=np.float32)
    beta = np.asarray(beta, dtype=np.float32)
    W_u = np.asarray(W_u, dtype=np.float32)
    W_i = np.asarray(W_i, dtype=np.float32)
    b_i = np.asarray(b_i, dtype=np.float32)
    w_e = np.asarray(w_e, dtype=np.float32)

    n = feat.shape[0]
    assert n == N_NODES and feat.shape[1] == D
    S = n // NCORES

    nc = _get_program(NCORES, S)
    in_maps = []
    for c in range(NCORES):
        sl = slice(c * S, (c + 1) * S)
        in_maps.append(
            _prep_core_inputs(feat[sl], pw[sl], W_u, W_i, b_i, w_e, gamma, beta)
        )
    import os
    trace = bool(int(os.environ.get("ATTN_TRACE", "0")))
    res = run_bass_kernel_spmd(nc, in_maps, list(range(NCORES)), trace=trace)
    global LAST_RESULT
    LAST_RESULT = res
    rst, pos = _finalize(res.results, NCORES, S, gamma, beta, pw)
    return rst, pos


# revision 15
# speedup vs baseline: 1.0994x; 1.0994x over previous
"""AttnReadout (segment softmax readout) Trainium2 kernel.

Math (reference):
  f = BN(feat) = feat*A + B        A = gamma*rsqrt(var+eps), B = beta-mean*A
  e = sigmoid(f@W_u.T + (f[last]@W_i.T + b_i)[seg]) @ w_e
  alpha = segment_softmax(e)
  rst = segsum(f*alpha);  position_rst = segsum(f*pw)

Device strategy (8 cores, node-dim sharded, 131072 nodes = 2048 segs/core):
  pass1 (native feat):  colsum, sumsq (PE ones-matmuls, PSUM accum),
                        P = segsum(pw*feat), spw = segsum(pw)  (PE pwblk matmuls)
  allreduce colsum/sumsq -> A,B on device -> fold into weights:
                        WuaT = A*W_u.T, WiaT = A*W_i.T, c = B@W_u.T+B@W_i.T+b_i
  pass2 (host-transposed featT, j-major cols: c = 128*j + g per supertile):
      zT = WuaT.T@featT + FL[seg]   (PE; FL added via periodic indicator matmul)
      t = tanh(0.5*zT + 0.5*c)      (ACT; sigmoid = .5+.5*tanh, const folds out
                                     of softmax so e' = (0.5*w_e)@t)
      e'-pack: per 128-node j-block, matmul(lhsT=th_block, rhs=we) -> one PSUM
               column => e' lands packed [128seg x 64j]; exp on packed (cheap)
      ex broadcast: PE-transpose -> row [1,8192] -> gpsimd partition_broadcast
      wft = ft*ex_rep (DVE 2x); segment sums via 6-level pairwise add tree
      (j-major makes tree levels contiguous)  -> sexT [f, g]
  host: rst = A*(sex/denom)+B ; position_rst = A*P + B*spw
"""

import numpy as np

N_NODES = 1048576
N_SEG = 16384
SEG = 64
D = 128
EPS = 1e-5
NCORES = 8

_CACHE = {}
LAST_RESULT = None  # BassKernelResults of the most recent kernel() call


def _build_program(n_cores, S):
    """Build + compile the per-core program. S = nodes per shard."""
    import concourse.bass as bass
    import concourse.tile as tile
    from concourse import bacc, mybir

    NSEGS = S // SEG          # segments per shard
    NSUP = S // 8192          # supertiles (128 segs each)
    assert S % 8192 == 0

    nc = bacc.Bacc(
        "TRN2",
        target_bir_lowering=False,
        debug=False,
        enable_asserts=False,
        num_devices=n_cores,
    )
    dtf = mybir.dt.float32
    dth = mybir.dt.bfloat16
    F32 = mybir.ActivationFunctionType

    featN = nc.dram_tensor("featN", [S, D], dth, kind="ExternalInput").ap()
    featT = nc.dram_tensor("featT", [D, S], dth, kind="ExternalInput").ap()
    lastT = nc.dram_tensor("lastT", [D, NSEGS], dth, kind="ExternalInput").ap()
    pwm = nc.dram_tensor("pwm", [D, S // D], dth, kind="ExternalInput").ap()
    wut = nc.dram_tensor("wut", [D, D], dtf, kind="ExternalInput").ap()
    wit = nc.dram_tensor("wit", [D, D], dtf, kind="ExternalInput").ap()
    smalls = nc.dram_tensor("smalls", [D, 4], dtf, kind="ExternalInput").ap()
    ind128 = nc.dram_tensor("ind128", [D, 512], dth, kind="ExternalInput").ap()
    ident = nc.dram_tensor("ident", [D, D], dtf, kind="ExternalInput").ap()
    identh = nc.dram_tensor("identh", [D, D], dth, kind="ExternalInput").ap()

    sexT = nc.dram_tensor("sexT", [D, NSEGS], dtf, kind="ExternalOutput").ap()
    den = nc.dram_tensor("den", [D, NSUP], dtf, kind="ExternalOutput").ap()
    poutT = nc.dram_tensor("poutT", [D, NSEGS], dtf, kind="ExternalOutput").ap()
    statsout = nc.dram_tensor("statsout", [D, 2], dtf, kind="ExternalOutput").ap()

    AL = mybir.AluOpType

    with tile.TileContext(nc) as tc:
        from contextlib import ExitStack

        with ExitStack() as ctx:
            singles = ctx.enter_context(tc.tile_pool(name="singles", bufs=1))

            wut_sb = singles.tile([D, D], dtf)
            nc.sync.dma_start(wut_sb[:], wut)
            wit_sb = singles.tile([D, D], dtf)
            nc.sync.dma_start(wit_sb[:], wit)
            smalls_sb = singles.tile([D, 4], dtf)
            nc.sync.dma_start(smalls_sb[:], smalls)
            ind128_sb = singles.tile([D, 512], dth)
            nc.sync.dma_start(ind128_sb[:], ind128)
            ident_sb = singles.tile([D, D], dtf)
            nc.sync.dma_start(ident_sb[:], ident)
            identh_sb = singles.tile([D, D], dth)
            nc.sync.dma_start(identh_sb[:], identh)
            ones_mat = singles.tile([D, D], dth)
            nc.vector.memset(ones_mat[:], 1.0)
            # rank-1 broadcast staging: row lives in partition 0, rest zero
            zrow = []
            for k in range(2):
                t = singles.tile([D, 8192], dth, tag=f"zrow{k}")
                nc.vector.memset(t[:], 0.0)
                zrow.append(t)
            gamma_c = smalls_sb[:, 0:1]
            beta_c = smalls_sb[:, 1:2]
            bi_c = smalls_sb[:, 2:3]
            we_c = smalls_sb[:, 3:4]
            we_half = singles.tile([D, 1], dtf)
            nc.vector.tensor_scalar_mul(we_half[:], we_c, 0.5)
            we_bf = singles.tile([D, 1], dth)
            nc.vector.tensor_copy(we_bf[:], we_half[:])
            bi_half = singles.tile([D, 1], dtf)
            nc.vector.tensor_scalar_mul(bi_half[:], bi_c, 0.5)

            # psum accumulators for global stats live through pass1+stats
            with tc.tile_pool(name="psacc", bufs=1, space="PSUM") as psacc:
             psum_gram = psacc.tile([D, D], dtf)    # feat.T@feat; diag = sumsq
             cs_accum = singles.tile([D, 1], dtf)
             nc.vector.memset(cs_accum[:], 0.0)

             # ---------------- PASS 1 : native layout ----------------
             with tc.tile_pool(name="p1in", bufs=3) as p1in, \
                  tc.tile_pool(name="p1pw", bufs=2) as p1pw, \
                  tc.tile_pool(name="pblk", bufs=1) as pblk, \
                  tc.tile_pool(name="psP", bufs=2, space="PSUM") as psP:

                 # combo3 per supertile: col 3t = pw upper half of tile t,
                 # 3t+1 = pw lower half, 3t+2 = ones (per-tile colsum lane)
                 combo = []
                 for k in range(2):
                     t = pblk.tile([D, 192], dth, tag=f"combo{k}")
                     nc.vector.memset(t[:], 0.0)
                     ap_ones = t[:, :].rearrange("p (t three) -> p t three",
                                                 three=3)[:, :, 2:3]
                     nc.vector.memset(ap_ones, 1.0)
                     combo.append(t)

                 for s in range(NSUP):
                     cmb = combo[s % 2]
                     pwm_st = p1pw.tile([D, SEG], dth)
                     nc.sync.dma_start(pwm_st[:], pwm[:, SEG * s:SEG * (s + 1)])
                     nc.vector.tensor_copy(
                         cmb[0:SEG, :].rearrange("p (t three) -> p t three",
                                                 three=3)[:, :, 0:1],
                         pwm_st[0:SEG, :],
                     )
                     nc.vector.tensor_copy(
                         cmb[SEG:D, :].rearrange("p (t three) -> p t three",
                                                 three=3)[:, :, 1:2],
                         pwm_st[SEG:D, :],
                     )

                     psum_PT = psP.tile([D, 192], dtf)
                     for b in range(2):  # big tiles: 4096 nodes each
                         base = 8192 * s + 4096 * b
                         ftn = p1in.tile([D, 32, D], dth)
                         nc.sync.dma_start(
                             ftn[:],
                             featN[base:base + 4096, :].rearrange(
                                 "(j p) d -> p j d", p=D
                             ),
                         )
                         first = (s == 0 and b == 0)
                         last = (s == NSUP - 1 and b == 1)
                         for j in range(32):  # 128-node tiles; ftn slice is lhsT
                             t_sup = 32 * b + j
                             lhs = ftn[:, j, :]
                             nc.tensor.matmul(
                                 psum_gram[:], lhs, lhs,
                                 start=(first and j == 0),
                                 stop=(last and j == 31),
                             )
                             mm = nc.tensor.matmul(
                                 psum_PT[:, 3 * t_sup:3 * t_sup + 3],
                                 lhs, cmb[:, 3 * t_sup:3 * t_sup + 3],
                                 start=True, stop=True,
                             )
                             mm.ins.ldweights = False
                     PT_sb = p1pw.tile([D, 192], dtf, tag="PT_sb")
                     nc.vector.tensor_copy(PT_sb[:], psum_PT[:])
                     # compact P pairs (cols 3t,3t+1) then one contiguous DMA
                     P_pack = p1pw.tile([D, D], dtf, tag="P_pack")
                     nc.vector.tensor_copy(
                         P_pack[:].rearrange("p (t two) -> p t two", two=2),
                         PT_sb[:].rearrange("p (t three) -> p t three",
                                            three=3)[:, :, 0:2],
                     )
                     nc.scalar.dma_start(poutT[:, D * s:D * (s + 1)], P_pack[:])
                     # colsum lanes (cols 3t+2) -> accumulate
                     csred = p1pw.tile([D, 1], dtf, tag="csred")
                     nc.vector.tensor_reduce(
                         csred[:],
                         PT_sb[:].rearrange("p (t three) -> p t three",
                                            three=3)[:, :, 2:3],
                         axis=mybir.AxisListType.XY, op=AL.add,
                     )
                     nc.vector.tensor_add(cs_accum[:], cs_accum[:], csred[:])

             # ---------------- STATS: allreduce + fold ----------------
             with tc.tile_pool(name="dram", bufs=1, space="DRAM") as dram, \
                  tc.tile_pool(name="stat", bufs=1) as stat, \
                  tc.tile_pool(name="psstat", bufs=1, space="PSUM") as psstat:
                 stats_col = stat.tile([D, 2], dtf)
                 nc.vector.tensor_copy(stats_col[:, 0:1], cs_accum[:])
                 # sumsq = diag(gram): mask with identity and row-reduce
                 gram_sb = stat.tile([D, D], dtf)
                 nc.vector.tensor_copy(gram_sb[:], psum_gram[:])
                 gmask = stat.tile([D, D], dtf)
                 nc.vector.tensor_mul(gmask[:], gram_sb[:], ident_sb[:])
                 nc.vector.tensor_reduce(
                     stats_col[:, 1:2], gmask[:],
                     axis=mybir.AxisListType.X, op=AL.add,
                 )
                 cc_in = dram.tile([D, 2], dtf)
                 cc_out = dram.tile([D, 2], dtf)
                 nc.sync.dma_start(cc_in[:], stats_col[:])
                 nc.gpsimd.collective_compute(
                     "AllReduce",
                     AL.add,
                     replica_groups=[list(range(n_cores))],
                     ins=[cc_in[:].opt()],
                     outs=[cc_out[:].opt()],
                 )
                 gstats = stat.tile([D, 2], dtf)
                 nc.sync.dma_start(gstats[:], cc_out[:])
                 nc.sync.dma_start(statsout, gstats[:])

                 n_tot = float(n_cores * S)
                 mean_c = stat.tile([D, 1], dtf)
                 nc.vector.tensor_scalar_mul(mean_c[:], gstats[:, 0:1], 1.0 / n_tot)
                 ex2_c = stat.tile([D, 1], dtf)
                 nc.vector.tensor_scalar_mul(ex2_c[:], gstats[:, 1:2], 1.0 / n_tot)
                 m2 = stat.tile([D, 1], dtf)
                 nc.vector.tensor_mul(m2[:], mean_c[:], mean_c[:])
                 var_c = stat.tile([D, 1], dtf)
                 nc.vector.tensor_sub(var_c[:], ex2_c[:], m2[:])
                 eps_t = stat.tile([D, 1], dtf)
                 nc.vector.memset(eps_t[:], EPS)
                 sd_c = stat.tile([D, 1], dtf)
                 nc.scalar.activation(sd_c[:], var_c[:], F32.Sqrt, bias=eps_t[:], scale=1.0)
                 rstd_c = stat.tile([D, 1], dtf)
                 nc.vector.reciprocal(rstd_c[:], sd_c[:])
                 A_c = stat.tile([D, 1], dtf)
                 nc.vector.tensor_mul(A_c[:], rstd_c[:], gamma_c)
                 mA = stat.tile([D, 1], dtf)
                 nc.vector.tensor_mul(mA[:], mean_c[:], A_c[:])
                 B_c = stat.tile([D, 1], dtf)
                 nc.vector.tensor_sub(B_c[:], beta_c, mA[:])

                 wuat = singles.tile([D, D], dth)
                 nc.vector.tensor_scalar_mul(wuat[:], wut_sb[:], A_c[:])
                 wiat = singles.tile([D, D], dth)
                 nc.vector.tensor_scalar_mul(wiat[:], wit_sb[:], A_c[:])
                 wsum = stat.tile([D, D], dtf)
                 nc.vector.tensor_add(wsum[:], wut_sb[:], wit_sb[:])
                 ps_c = psstat.tile([D, 1], dtf)
                 nc.tensor.matmul(ps_c[:], wsum[:], B_c[:], start=True, stop=True)
                 c_half = singles.tile([D, 1], dtf)
                 nc.vector.scalar_tensor_tensor(
                     c_half[:], ps_c[:], 0.5, bi_half[:], AL.mult, AL.add
                 )

            # ---------------- PASS 2 : transposed layout (j-major) ----------
            # fl pre-pass: FL[g,h] for all segs (frees psum banks for main loop)
            fl_all = singles.tile([D, NSUP, D], dth)
            with tc.tile_pool(name="flpre", bufs=2) as flprep, \
                 tc.tile_pool(name="psflp", bufs=2, space="PSUM") as psflp:
                for s in range(NSUP):
                    lt = flprep.tile([D, D], dth)
                    nc.sync.dma_start(lt[:], lastT[:, D * s:D * (s + 1)])
                    psum_fl = psflp.tile([D, D], dtf)
                    nc.tensor.matmul(psum_fl[:], lt[:], wiat[:], start=True, stop=True)
                    nc.vector.tensor_copy(fl_all[:, s, :], psum_fl[:])

            with tc.tile_pool(name="ft", bufs=2) as ftp, \
                 tc.tile_pool(name="th", bufs=3) as thp, \
                 tc.tile_pool(name="expk", bufs=2) as expkp, \
                 tc.tile_pool(name="ext", bufs=2) as extp, \
                 tc.tile_pool(name="row", bufs=2) as rowp, \
                 tc.tile_pool(name="exr", bufs=2) as exrp, \
                 tc.tile_pool(name="wft", bufs=2) as wftp, \
                 tc.tile_pool(name="tre", bufs=2) as trep, \
                 tc.tile_pool(name="sout", bufs=2) as soutp, \
                 tc.tile_pool(name="denall", bufs=1) as denallp, \
                 tc.tile_pool(name="psz", bufs=2, space="PSUM") as psz, \
                 tc.tile_pool(name="pse", bufs=1, space="PSUM") as pse, \
                 tc.tile_pool(name="ptr", bufs=1, space="PSUM") as ptrp, \
                 tc.tile_pool(name="pbc", bufs=2, space="PSUM") as pbcp:

                den_all = denallp.tile([D, NSUP], dtf)

                for s in range(NSUP):
                    ft = ftp.tile([D, 8192], dth)
                    nc.sync.dma_start(ft[:], featT[:, 8192 * s:8192 * (s + 1)])

                    psum_e = pse.tile([D, SEG], dtf)   # e' packed [g, j]
                    # phase A: scores; 8 psum tiles of 1024 cols (8 j-blocks)
                    for t in range(8):
                        psum_z = psz.tile([D, 1024], dtf)
                        for m in range(2):
                            mm = nc.tensor.matmul(
                                psum_z[:, 512 * m:512 * (m + 1)],
                                wuat[:],
                                ft[:, 1024 * t + 512 * m:1024 * t + 512 * (m + 1)],
                                start=True, stop=False,
                            )
                            if m == 1:
                                mm.ins.ldweights = False
                        for m in range(2):
                            mm = nc.tensor.matmul(
                                psum_z[:, 512 * m:512 * (m + 1)],
                                fl_all[:, s, :],
                                ind128_sb[:],
                                start=False, stop=True,
                            )
                            if m == 1:
                                mm.ins.ldweights = False
                        th_t = thp.tile([D, 1024], dth)
                        nc.scalar.activation(
                            th_t[:], psum_z[:], F32.Tanh, bias=c_half[:], scale=0.5
                        )
                        # e'-pack: one PSUM column per 128-node j-block
                        for b in range(8):
                            jj = 8 * t + b
                            nc.tensor.matmul(
                                psum_e[:, jj:jj + 1],
                                th_t[:, D * b:D * (b + 1)],
                                we_bf[:],
                                start=True, stop=True,
                            )
                    # exp on packed + denominators
                    ex_pk = expkp.tile([D, SEG], dth)
                    nc.scalar.activation(ex_pk[:], psum_e[:], F32.Exp)
                    nc.vector.tensor_reduce(
                        den_all[:, s:s + 1], ex_pk[:],
                        axis=mybir.AxisListType.X, op=AL.add,
                    )
                    # broadcast ex to [f, c]: transpose -> row -> bcast-AP DMA
                    # (scalar HWDGE queue: keeps the sync queue free for the
                    # ft prefetches, which would otherwise block behind these)
                    ptr_t = ptrp.tile([SEG, D], dth)
                    nc.tensor.transpose(ptr_t[:], ex_pk[:], identh_sb[:])
                    exT = extp.tile([SEG, D], dth)
                    nc.vector.tensor_copy(exT[:], ptr_t[:])
                    row = zrow[s % 2]
                    nc.scalar.dma_start(
                        row[0:1, :].rearrange("o (j g) -> o j g", j=SEG), exT[:]
                    )
                    # PE rank-1 broadcast (ones.T @ [row; 0...]) -> PSUM,
                    # engine-copied to SBUF (no SDMA traffic), then DVE mul.
                    ex_rep = exrp.tile([D, 8192], dth)
                    wft = wftp.tile([D, 8192], dth)
                    for h in range(16):
                        sl = slice(512 * h, 512 * (h + 1))
                        pbc = pbcp.tile([D, 512], dtf)
                        nc.tensor.matmul(
                            pbc[:], ones_mat[:], row[:, sl],
                            start=True, stop=True,
                        )
                        if h % 2 == 0:
                            nc.scalar.copy(ex_rep[:, sl], pbc[:])
                        else:
                            nc.vector.tensor_copy(ex_rep[:, sl], pbc[:])
                    for q in range(8):
                        sl = slice(1024 * q, 1024 * (q + 1))
                        nc.vector.tensor_mul(
                            wft[:, sl], ft[:, sl], ex_rep[:, sl]
                        )
                    t1 = trep.tile([D, 4096], dth, tag="t1")
                    nc.vector.tensor_add(t1[:], wft[:, 0:4096], wft[:, 4096:8192])
                    t2 = trep.tile([D, 2048], dth, tag="t2")
                    nc.vector.tensor_add(t2[:], t1[:, 0:2048], t1[:, 2048:4096])
                    t3 = trep.tile([D, 1024], dth, tag="t3")
                    nc.vector.tensor_add(t3[:], t2[:, 0:1024], t2[:, 1024:2048])
                    t4 = trep.tile([D, 512], dth, tag="t4")
                    nc.vector.tensor_add(t4[:], t3[:, 0:512], t3[:, 512:1024])
                    t5 = trep.tile([D, 256], dtf, tag="t5")
                    nc.vector.tensor_add(t5[:], t4[:, 0:256], t4[:, 256:512])
                    sexG = soutp.tile([D, D], dtf)
                    nc.vector.tensor_add(sexG[:], t5[:, 0:128], t5[:, 128:256])
                    nc.scalar.dma_start(sexT[:, D * s:D * (s + 1)], sexG[:])

                nc.scalar.dma_start(den, den_all[:])

    nc.compile()
    return nc


def _get_program(n_cores, S):
    key = (n_cores, S)
    if key not in _CACHE:
        _CACHE[key] = _build_program(n_cores, S)
    return _CACHE[key]


def _prep_core_inputs(feat_sh, pw_sh, W_u, W_i, b_i, w_e, gamma, beta):
    S = feat_sh.shape[0]
    NSUP = S // 8192
    import ml_dtypes
    f16 = ml_dtypes.bfloat16
    # j-major transposed layout: col (s, j, g) = node 8192*s + 64*g + j
    featT = np.ascontiguousarray(
        feat_sh.reshape(NSUP, 128, SEG, D).transpose(3, 0, 2, 1).reshape(D, S)
    ).astype(f16)
    lastT = np.ascontiguousarray(feat_sh[SEG - 1::SEG, :].T).astype(f16)
    pwm = np.ascontiguousarray(pw_sh.reshape(-1, D).T).astype(f16)
    # FL periodic indicator: ind128[g, c] = 1 iff c % 128 == g
    ind128 = np.tile(np.eye(D, dtype=np.float32), (1, 4)).astype(f16)
    smalls = np.stack([gamma, beta, b_i, w_e], axis=1).astype(np.float32)
    return {
        "featN": np.ascontiguousarray(feat_sh).astype(f16),
        "featT": featT,
        "lastT": lastT,
        "pwm": pwm,
        "wut": np.ascontiguousarray(W_u.T),
        "wit": np.ascontiguousarray(W_i.T),
        "smalls": smalls,
        "ind128": ind128,
        "ident": np.eye(D, dtype=np.float32),
        "identh": np.eye(D, dtype=np.float32).astype(f16),
    }


def _finalize(results, n_cores, S, gamma, beta, pw):
    NSEGS = S // SEG
    spw_all = pw.astype(np.float64).reshape(-1, SEG).sum(1).astype(np.float32)
    st = results[0]["statsout"]            # [D, 2]
    n_tot = float(n_cores * S)
    mean = st[:, 0] / n_tot
    var = st[:, 1] / n_tot - mean * mean
    A = gamma / np.sqrt(var + EPS)
    B = beta - mean * A
    rst = np.empty((n_cores * NSEGS, D), dtype=np.float32)
    pos = np.empty((n_cores * NSEGS, D), dtype=np.float32)
    for c in range(n_cores):
        r = results[c]
        sex = r["sexT"].T                      # [NSEGS, D]
        denom = r["den"].T.reshape(-1)         # seg order
        p = r["poutT"].T                       # [NSEGS, D]
        sl = slice(c * NSEGS, (c + 1) * NSEGS)
        spw = spw_all[sl]
        rst[sl] = A * (sex / denom[:, None]) + B
        pos[sl] = A * p + B * spw[:, None]
    return rst, pos


def kernel(feat, position_weight, last_nodes, segment_ids, gamma, beta,
           W_u, W_i, b_i, w_e, num_segments):
    from concourse.bass_utils import run_bass_kernel_spmd

    feat = np.asarray(feat, dtype=np.float32)
    pw = np.asarray(position_weight, dtype=np.float32)
    gamma = np.asarray(gamma, dtype=np.float32)
    beta = np.asarray(beta, dtype=np.float32)
    W_u = np.asarray(W_u, dtype=np.float32)
    W_i = np.asarray(W_i, dtype=np.float32)
    b_i = np.asarray(b_i, dtype=np.float32)
    w_e = np.asarray(w_e, dtype=np.float32)

    n = feat.shape[0]
    assert n == N_NODES and feat.shape[1] == D
    S = n // NCORES

    nc = _get_program(NCORES, S)
    in_maps = []
    for c in range(NCORES):
        sl = slice(c * S, (c + 1) * S)
        in_maps.append(
            _prep_core_inputs(feat[sl], pw[sl], W_u, W_i, b_i, w_e, gamma, beta)
        )
    import os
    trace = bool(int(os.environ.get("ATTN_TRACE", "0")))
    res = run_bass_kernel_spmd(nc, in_maps, list(range(NCORES)), trace=trace)
    global LAST_RESULT
    LAST_RESULT = res
    rst, pos = _finalize(res.results, NCORES, S, gamma, beta, pw)
    return rst, pos


# revision 19
# speedup vs baseline: 1.2074x; 1.0982x over previous
"""AttnReadout (segment softmax readout) Trainium2 kernel.

Math (reference):
  f = BN(feat) = feat*A + B        A = gamma*rsqrt(var+eps), B = beta-mean*A
  e = sigmoid(f@W_u.T + (f[last]@W_i.T + b_i)[seg]) @ w_e
  alpha = segment_softmax(e)
  rst = segsum(f*alpha);  position_rst = segsum(f*pw)

Device strategy (8 cores, node-dim sharded, 131072 nodes = 2048 segs/core):
  pass1 (native feat):  colsum, sumsq (PE ones-matmuls, PSUM accum),
                        P = segsum(pw*feat), spw = segsum(pw)  (PE pwblk matmuls)
  allreduce colsum/sumsq -> A,B on device -> fold into weights:
                        WuaT = A*W_u.T, WiaT = A*W_i.T, c = B@W_u.T+B@W_i.T+b_i
  pass2 (host-transposed featT, j-major cols: c = 128*j + g per supertile):
      zT = WuaT.T@featT + FL[seg]   (PE; FL added via periodic indicator matmul)
      t = tanh(0.5*zT + 0.5*c)      (ACT; sigmoid = .5+.5*tanh, const folds out
                                     of softmax so e' = (0.5*w_e)@t)
      e'-pack: per 128-node j-block, matmul(lhsT=th_block, rhs=we) -> one PSUM
               column => e' lands packed [128seg x 64j]; exp on packed (cheap)
      ex broadcast: PE-transpose -> row [1,8192] -> gpsimd partition_broadcast
      wft = ft*ex_rep (DVE 2x); segment sums via 6-level pairwise add tree
      (j-major makes tree levels contiguous)  -> sexT [f, g]
  host: rst = A*(sex/denom)+B ; position_rst = A*P + B*spw
"""

import numpy as np

N_NODES = 1048576
N_SEG = 16384
SEG = 64
D = 128
EPS = 1e-5
NCORES = 8

_CACHE = {}
LAST_RESULT = None  # BassKernelResults of the most recent kernel() call


def _build_program(n_cores, S):
    """Build + compile the per-core program. S = nodes per shard."""
    import concourse.bass as bass
    import concourse.tile as tile
    from concourse import bacc, mybir

    NSEGS = S // SEG          # segments per shard
    NSUP = S // 8192          # supertiles (128 segs each)
    assert S % 8192 == 0

    nc = bacc.Bacc(
        "TRN2",
        target_bir_lowering=False,
        debug=False,
        enable_asserts=False,
        num_devices=n_cores,
    )
    dtf = mybir.dt.float32
    dth = mybir.dt.bfloat16
    F32 = mybir.ActivationFunctionType

    featN = nc.dram_tensor("featN", [S, D], dth, kind="ExternalInput").ap()
    featT = nc.dram_tensor("featT", [D, S], dth, kind="ExternalInput").ap()
    lastT = nc.dram_tensor("lastT", [D, NSEGS], dth, kind="ExternalInput").ap()
    pwm = nc.dram_tensor("pwm", [D, S // D], dth, kind="ExternalInput").ap()
    wut = nc.dram_tensor("wut", [D, D], dtf, kind="ExternalInput").ap()
    wit = nc.dram_tensor("wit", [D, D], dtf, kind="ExternalInput").ap()
    smalls = nc.dram_tensor("smalls", [D, 4], dtf, kind="ExternalInput").ap()
    ind128 = nc.dram_tensor("ind128", [D, 512], dth, kind="ExternalInput").ap()
    ident = nc.dram_tensor("ident", [D, D], dtf, kind="ExternalInput").ap()
    identh = nc.dram_tensor("identh", [D, D], dth, kind="ExternalInput").ap()

    sexT = nc.dram_tensor("sexT", [D, NSEGS], dtf, kind="ExternalOutput").ap()
    den = nc.dram_tensor("den", [D, NSUP], dtf, kind="ExternalOutput").ap()
    poutT = nc.dram_tensor("poutT", [D, NSEGS], dtf, kind="ExternalOutput").ap()
    statsout = nc.dram_tensor("statsout", [D, 2], dtf, kind="ExternalOutput").ap()

    AL = mybir.AluOpType

    with tile.TileContext(nc) as tc:
        from contextlib import ExitStack

        with ExitStack() as ctx:
            singles = ctx.enter_context(tc.tile_pool(name="singles", bufs=1))

            wut_sb = singles.tile([D, D], dtf)
            nc.sync.dma_start(wut_sb[:], wut)
            wit_sb = singles.tile([D, D], dtf)
            nc.sync.dma_start(wit_sb[:], wit)
            smalls_sb = singles.tile([D, 4], dtf)
            nc.sync.dma_start(smalls_sb[:], smalls)
            ind128_sb = singles.tile([D, 512], dth)
            nc.sync.dma_start(ind128_sb[:], ind128)
            ident_sb = singles.tile([D, D], dtf)
            nc.sync.dma_start(ident_sb[:], ident)
            identh_sb = singles.tile([D, D], dth)
            nc.sync.dma_start(identh_sb[:], identh)
            ones_mat = singles.tile([D, D], dth)
            nc.vector.memset(ones_mat[:], 1.0)
            # rank-1 broadcast staging: row lives in partition 0, rest zero
            zrow = []
            for k in range(2):
                t = singles.tile([D, 8192], dth, tag=f"zrow{k}")
                nc.vector.memset(t[:], 0.0)
                zrow.append(t)
            gamma_c = smalls_sb[:, 0:1]
            beta_c = smalls_sb[:, 1:2]
            bi_c = smalls_sb[:, 2:3]
            we_c = smalls_sb[:, 3:4]
            we_half = singles.tile([D, 1], dtf)
            nc.vector.tensor_scalar_mul(we_half[:], we_c, 0.5)
            we_bf = singles.tile([D, 1], dth)
            nc.vector.tensor_copy(we_bf[:], we_half[:])
            bi_half = singles.tile([D, 1], dtf)
            nc.vector.tensor_scalar_mul(bi_half[:], bi_c, 0.5)

            # psum accumulators for global stats live through pass1+stats
            with tc.tile_pool(name="psacc", bufs=1, space="PSUM") as psacc:
             psum_gram = psacc.tile([D, D], dtf)    # feat.T@feat; diag = sumsq
             cs_accum = singles.tile([D, 1], dtf)
             nc.vector.memset(cs_accum[:], 0.0)

             # ---------------- PASS 1 : native layout ----------------
             with tc.tile_pool(name="p1in", bufs=3) as p1in, \
                  tc.tile_pool(name="p1pw", bufs=2) as p1pw, \
                  tc.tile_pool(name="pblk", bufs=1) as pblk, \
                  tc.tile_pool(name="psP", bufs=2, space="PSUM") as psP:

                 # combo3 per supertile: col 3t = pw upper half of tile t,
                 # 3t+1 = pw lower half, 3t+2 = ones (per-tile colsum lane)
                 combo = []
                 for k in range(2):
                     t = pblk.tile([D, 192], dth, tag=f"combo{k}")
                     nc.vector.memset(t[:], 0.0)
                     ap_ones = t[:, :].rearrange("p (t three) -> p t three",
                                                 three=3)[:, :, 2:3]
                     nc.vector.memset(ap_ones, 1.0)
                     combo.append(t)

                 for s in range(NSUP):
                     cmb = combo[s % 2]
                     pwm_st = p1pw.tile([D, SEG], dth)
                     nc.scalar.dma_start(pwm_st[:], pwm[:, SEG * s:SEG * (s + 1)])
                     nc.vector.tensor_copy(
                         cmb[0:SEG, :].rearrange("p (t three) -> p t three",
                                                 three=3)[:, :, 0:1],
                         pwm_st[0:SEG, :],
                     )
                     nc.vector.tensor_copy(
                         cmb[SEG:D, :].rearrange("p (t three) -> p t three",
                                                 three=3)[:, :, 1:2],
                         pwm_st[SEG:D, :],
                     )

                     psum_PT = psP.tile([D, 192], dtf)
                     for b in range(2):  # big tiles: 4096 nodes each
                         base = 8192 * s + 4096 * b
                         ftn = p1in.tile([D, 32, D], dth)
                         nc.sync.dma_start(
                             ftn[:],
                             featN[base:base + 4096, :].rearrange(
                                 "(j p) d -> p j d", p=D
                             ),
                         )
                         first = (s == 0 and b == 0)
                         last = (s == NSUP - 1 and b == 1)
                         for j in range(32):  # 128-node tiles; ftn slice is lhsT
                             t_sup = 32 * b + j
                             lhs = ftn[:, j, :]
                             nc.tensor.matmul(
                                 psum_gram[:], lhs, lhs,
                                 start=(first and j == 0),
                                 stop=(last and j == 31),
                             )
                             mm = nc.tensor.matmul(
                                 psum_PT[:, 3 * t_sup:3 * t_sup + 3],
                                 lhs, cmb[:, 3 * t_sup:3 * t_sup + 3],
                                 start=True, stop=True,
                             )
                             mm.ins.ldweights = False
                     PT_sb = p1pw.tile([D, 192], dtf, tag="PT_sb")
                     nc.vector.tensor_copy(PT_sb[:], psum_PT[:])
                     # compact P pairs (cols 3t,3t+1) then one contiguous DMA
                     P_pack = p1pw.tile([D, D], dtf, tag="P_pack")
                     nc.vector.tensor_copy(
                         P_pack[:].rearrange("p (t two) -> p t two", two=2),
                         PT_sb[:].rearrange("p (t three) -> p t three",
                                            three=3)[:, :, 0:2],
                     )
                     nc.scalar.dma_start(poutT[:, D * s:D * (s + 1)], P_pack[:])
                     # colsum lanes (cols 3t+2) -> accumulate
                     csred = p1pw.tile([D, 1], dtf, tag="csred")
                     nc.vector.tensor_reduce(
                         csred[:],
                         PT_sb[:].rearrange("p (t three) -> p t three",
                                            three=3)[:, :, 2:3],
                         axis=mybir.AxisListType.XY, op=AL.add,
                     )
                     nc.vector.tensor_add(cs_accum[:], cs_accum[:], csred[:])

             # ---------------- STATS: allreduce + fold ----------------
             with tc.tile_pool(name="dram", bufs=1, space="DRAM") as dram, \
                  tc.tile_pool(name="stat", bufs=1) as stat, \
                  tc.tile_pool(name="psstat", bufs=1, space="PSUM") as psstat:
                 stats_col = stat.tile([D, 2], dtf)
                 nc.vector.tensor_copy(stats_col[:, 0:1], cs_accum[:])
                 # sumsq = diag(gram): mask with identity and row-reduce
                 gram_sb = stat.tile([D, D], dtf)
                 nc.vector.tensor_copy(gram_sb[:], psum_gram[:])
                 gmask = stat.tile([D, D], dtf)
                 nc.vector.tensor_mul(gmask[:], gram_sb[:], ident_sb[:])
                 nc.vector.tensor_reduce(
                     stats_col[:, 1:2], gmask[:],
                     axis=mybir.AxisListType.X, op=AL.add,
                 )
                 cc_in = dram.tile([D, 2], dtf)
                 cc_out = dram.tile([D, 2], dtf)
                 nc.sync.dma_start(cc_in[:], stats_col[:])
                 nc.gpsimd.collective_compute(
                     "AllReduce",
                     AL.add,
                     replica_groups=[list(range(n_cores))],
                     ins=[cc_in[:].opt()],
                     outs=[cc_out[:].opt()],
                 )
                 gstats = stat.tile([D, 2], dtf)
                 nc.sync.dma_start(gstats[:], cc_out[:])
                 nc.sync.dma_start(statsout, gstats[:])

                 n_tot = float(n_cores * S)
                 mean_c = stat.tile([D, 1], dtf)
                 nc.vector.tensor_scalar_mul(mean_c[:], gstats[:, 0:1], 1.0 / n_tot)
                 ex2_c = stat.tile([D, 1], dtf)
                 nc.vector.tensor_scalar_mul(ex2_c[:], gstats[:, 1:2], 1.0 / n_tot)
                 m2 = stat.tile([D, 1], dtf)
                 nc.vector.tensor_mul(m2[:], mean_c[:], mean_c[:])
                 var_c = stat.tile([D, 1], dtf)
                 nc.vector.tensor_sub(var_c[:], ex2_c[:], m2[:])
                 eps_t = stat.tile([D, 1], dtf)
                 nc.vector.memset(eps_t[:], EPS)
                 sd_c = stat.tile([D, 1], dtf)
                 nc.scalar.activation(sd_c[:], var_c[:], F32.Sqrt, bias=eps_t[:], scale=1.0)
                 rstd_c = stat.tile([D, 1], dtf)
                 nc.vector.reciprocal(rstd_c[:], sd_c[:])
                 A_c = stat.tile([D, 1], dtf)
                 nc.vector.tensor_mul(A_c[:], rstd_c[:], gamma_c)
                 mA = stat.tile([D, 1], dtf)
                 nc.vector.tensor_mul(mA[:], mean_c[:], A_c[:])
                 B_c = stat.tile([D, 1], dtf)
                 nc.vector.tensor_sub(B_c[:], beta_c, mA[:])

                 wuat = singles.tile([D, D], dth)
                 nc.vector.tensor_scalar_mul(wuat[:], wut_sb[:], A_c[:])
                 wiat = singles.tile([D, D], dth)
                 nc.vector.tensor_scalar_mul(wiat[:], wit_sb[:], A_c[:])
                 wsum = stat.tile([D, D], dtf)
                 nc.vector.tensor_add(wsum[:], wut_sb[:], wit_sb[:])
                 ps_c = psstat.tile([D, 1], dtf)
                 nc.tensor.matmul(ps_c[:], wsum[:], B_c[:], start=True, stop=True)
                 c_half = singles.tile([D, 1], dtf)
                 nc.vector.scalar_tensor_tensor(
                     c_half[:], ps_c[:], 0.5, bi_half[:], AL.mult, AL.add
                 )

            # ---------------- PASS 2 : transposed layout (j-major) ----------
            # fl pre-pass: FL[g,h] for all segs (frees psum banks for main loop)
            fl_all = singles.tile([D, NSUP, D], dth)
            with tc.tile_pool(name="flpre", bufs=2) as flprep, \
                 tc.tile_pool(name="psflp", bufs=2, space="PSUM") as psflp:
                for s in range(NSUP):
                    lt = flprep.tile([D, D], dth)
                    nc.scalar.dma_start(lt[:], lastT[:, D * s:D * (s + 1)])
                    psum_fl = psflp.tile([D, D], dtf)
                    nc.tensor.matmul(psum_fl[:], lt[:], wiat[:], start=True, stop=True)
                    nc.vector.tensor_copy(fl_all[:, s, :], psum_fl[:])

            with tc.tile_pool(name="ft", bufs=4) as ftp, \
                 tc.tile_pool(name="th", bufs=3) as thp, \
                 tc.tile_pool(name="expk", bufs=2) as expkp, \
                 tc.tile_pool(name="ext", bufs=2) as extp, \
                 tc.tile_pool(name="exr", bufs=2) as exrp, \
                 tc.tile_pool(name="sout", bufs=2) as soutp, \
                 tc.tile_pool(name="denall", bufs=1) as denallp, \
                 tc.tile_pool(name="psz", bufs=2, space="PSUM") as psz, \
                 tc.tile_pool(name="pse", bufs=1, space="PSUM") as pse, \
                 tc.tile_pool(name="ptr", bufs=1, space="PSUM") as ptrp, \
                 tc.tile_pool(name="pbc", bufs=2, space="PSUM") as pbcp:

                den_all = denallp.tile([D, NSUP], dtf)

                for s in range(NSUP):
                    ft = ftp.tile([D, 8192], dth)
                    nc.sync.dma_start(ft[:], featT[:, 8192 * s:8192 * (s + 1)])

                    psum_e = pse.tile([D, SEG], dtf)   # e' packed [g, j]
                    # phase A: scores; 8 psum tiles of 1024 cols (8 j-blocks)
                    for t in range(8):
                        psum_z = psz.tile([D, 1024], dtf)
                        for m in range(2):
                            mm = nc.tensor.matmul(
                                psum_z[:, 512 * m:512 * (m + 1)],
                                wuat[:],
                                ft[:, 1024 * t + 512 * m:1024 * t + 512 * (m + 1)],
                                start=True, stop=False,
                            )
                            if m == 1:
                                mm.ins.ldweights = False
                        for m in range(2):
                            mm = nc.tensor.matmul(
                                psum_z[:, 512 * m:512 * (m + 1)],
                                fl_all[:, s, :],
                                ind128_sb[:],
                                start=False, stop=True,
                            )
                            if m == 1:
                                mm.ins.ldweights = False
                        th_t = thp.tile([D, 1024], dth)
                        nc.scalar.activation(
                            th_t[:], psum_z[:], F32.Tanh, bias=c_half[:], scale=0.5
                        )
                        # e'-pack: one PSUM column per 128-node j-block
                        for b in range(8):
                            jj = 8 * t + b
                            nc.tensor.matmul(
                                psum_e[:, jj:jj + 1],
                                th_t[:, D * b:D * (b + 1)],
                                we_bf[:],
                                start=True, stop=True,
                            )
                    # exp on packed + denominators
                    ex_pk = expkp.tile([D, SEG], dth)
                    nc.scalar.activation(ex_pk[:], psum_e[:], F32.Exp)
                    nc.vector.tensor_reduce(
                        den_all[:, s:s + 1], ex_pk[:],
                        axis=mybir.AxisListType.X, op=AL.add,
                    )
                    # broadcast ex to [f, c]: transpose -> row -> bcast-AP DMA
                    # (scalar HWDGE queue: keeps the sync queue free for the
                    # ft prefetches, which would otherwise block behind these)
                    ptr_t = ptrp.tile([SEG, D], dth)
                    nc.tensor.transpose(ptr_t[:], ex_pk[:], identh_sb[:])
                    exT = extp.tile([SEG, D], dth)
                    nc.vector.tensor_copy(exT[:], ptr_t[:])
                    row = zrow[s % 2]
                    nc.scalar.dma_start(
                        row[0:1, :].rearrange("o (j g) -> o j g", j=SEG), exT[:]
                    )
                    # PE rank-1 broadcast (ones.T @ [row; 0...]) -> PSUM,
                    # engine-copied to SBUF (no SDMA traffic), then DVE mul.
                    ex_rep = exrp.tile([D, 8192], dth)
                    for h in range(16):
                        sl = slice(512 * h, 512 * (h + 1))
                        pbc = pbcp.tile([D, 512], dtf)
                        nc.tensor.matmul(
                            pbc[:], ones_mat[:], row[:, sl],
                            start=True, stop=True,
                        )
                        if h % 2 == 0:
                            nc.scalar.copy(ex_rep[:, sl], pbc[:])
                        else:
                            nc.vector.tensor_copy(ex_rep[:, sl], pbc[:])
                    # weighted features in place (ft's last use), then the
                    # in-place pairwise tree (j-major keeps levels contiguous)
                    for q in range(8):
                        sl = slice(1024 * q, 1024 * (q + 1))
                        nc.vector.tensor_mul(
                            ft[:, sl], ft[:, sl], ex_rep[:, sl]
                        )
                    nc.vector.tensor_add(ft[:, 0:4096], ft[:, 0:4096],
                                         ft[:, 4096:8192])
                    nc.vector.tensor_add(ft[:, 0:2048], ft[:, 0:2048],
                                         ft[:, 2048:4096])
                    nc.vector.tensor_add(ft[:, 0:1024], ft[:, 0:1024],
                                         ft[:, 1024:2048])
                    nc.vector.tensor_add(ft[:, 0:512], ft[:, 0:512],
                                         ft[:, 512:1024])
                    t5 = soutp.tile([D, 256], dtf, tag="t5")
                    nc.vector.tensor_add(t5[:], ft[:, 0:256], ft[:, 256:512])
                    sexG = soutp.tile([D, D], dtf)
                    nc.vector.tensor_add(sexG[:], t5[:, 0:128], t5[:, 128:256])
                    nc.scalar.dma_start(sexT[:, D * s:D * (s + 1)], sexG[:])

                nc.scalar.dma_start(den, den_all[:])

    nc.compile()
    return nc


def _get_program(n_cores, S):
    key = (n_cores, S)
    if key not in _CACHE:
        _CACHE[key] = _build_program(n_cores, S)
    return _CACHE[key]


def _prep_core_inputs(feat_sh, pw_sh, W_u, W_i, b_i, w_e, gamma, beta):
    S = feat_sh.shape[0]
    NSUP = S // 8192
    import ml_dtypes
    f16 = ml_dtypes.bfloat16
    # j-major transposed layout: col (s, j, g) = node 8192*s + 64*g + j
    featT = np.ascontiguousarray(
        feat_sh.reshape(NSUP, 128, SEG, D).transpose(3, 0, 2, 1).reshape(D, S)
    ).astype(f16)
    lastT = np.ascontiguousarray(feat_sh[SEG - 1::SEG, :].T).astype(f16)
    pwm = np.ascontiguousarray(pw_sh.reshape(-1, D).T).astype(f16)
    # FL periodic indicator: ind128[g, c] = 1 iff c % 128 == g
    ind128 = np.tile(np.eye(D, dtype=np.float32), (1, 4)).astype(f16)
    smalls = np.stack([gamma, beta, b_i, w_e], axis=1).astype(np.float32)
    return {
        "featN": np.ascontiguousarray(feat_sh).astype(f16),
        "featT": featT,
        "lastT": lastT,
        "pwm": pwm,
        "wut": np.ascontiguousarray(W_u.T),
        "wit": np.ascontiguousarray(W_i.T),
        "smalls": smalls,
        "ind128": ind128,
        "ident": np.eye(D, dtype=np.float32),
        "identh": np.eye(D, dtype=np.float32).astype(f16),
    }


def _finalize(results, n_cores, S, gamma, beta, pw):
    NSEGS = S // SEG
    spw_all = pw.astype(np.float64).reshape(-1, SEG).sum(1).astype(np.float32)
    st = results[0]["statsout"]            # [D, 2]
    n_tot = float(n_cores * S)
    mean = st[:, 0] / n_tot
    var = st[:, 1] / n_tot - mean * mean
    A = gamma / np.sqrt(var + EPS)
    B = beta - mean * A
    rst = np.empty((n_cores * NSEGS, D), dtype=np.float32)
    pos = np.empty((n_cores * NSEGS, D), dtype=np.float32)
    for c in range(n_cores):
        r = results[c]
        sex = r["sexT"].T                      # [NSEGS, D]
        denom = r["den"].T.reshape(-1)         # seg order
        p = r["poutT"].T                       # [NSEGS, D]
        sl = slice(c * NSEGS, (c + 1) * NSEGS)
        spw = spw_all[sl]
        rst[sl] = A * (sex / denom[:, None]) + B
        pos[sl] = A * p + B * spw[:, None]
    return rst, pos


def kernel(feat, position_weight, last_nodes, segment_ids, gamma, beta,
           W_u, W_i, b_i, w_e, num_segments):
    from concourse.bass_utils import run_bass_kernel_spmd

    feat = np.asarray(feat, dtype=np.float32)
    pw = np.asarray(position_weight, dtype=np.float32)
    gamma = np.asarray(gamma, dtype=np.float32)
    beta = np.asarray(beta, dtype=np.float32)
    W_u = np.asarray(W_u, dtype=np.float32)
    W_i = np.asarray(W_i, dtype=np.float32)
    b_i = np.asarray(b_i, dtype=np.float32)
    w_e = np.asarray(w_e, dtype=np.float32)

    n = feat.shape[0]
    assert n == N_NODES and feat.shape[1] == D
    S = n // NCORES

    nc = _get_program(NCORES, S)
    in_maps = []
    for c in range(NCORES):
        sl = slice(c * S, (c + 1) * S)
        in_maps.append(
            _prep_core_inputs(feat[sl], pw[sl], W_u, W_i, b_i, w_e, gamma, beta)
        )
    import os
    trace = bool(int(os.environ.get("ATTN_TRACE", "0")))
    res = run_bass_kernel_spmd(nc, in_maps, list(range(NCORES)), trace=trace)
    global LAST_RESULT
    LAST_RESULT = res
    rst, pos = _finalize(res.results, NCORES, S, gamma, beta, pw)
    return rst, pos


# revision 26
# speedup vs baseline: 1.4692x; 1.2168x over previous
"""AttnReadout (segment softmax readout) Trainium2 kernel.

Math (reference):
  f = BN(feat) = feat*A + B        A = gamma*rsqrt(var+eps), B = beta-mean*A
  e = sigmoid(f@W_u.T + (f[last]@W_i.T + b_i)[seg]) @ w_e
  alpha = segment_softmax(e)
  rst = segsum(f*alpha);  position_rst = segsum(f*pw)

Device strategy (8 cores, node-dim sharded, 131072 nodes = 2048 segs/core):
  pass1 (native feat):  colsum, sumsq (PE ones-matmuls, PSUM accum),
                        P = segsum(pw*feat), spw = segsum(pw)  (PE pwblk matmuls)
  allreduce colsum/sumsq -> A,B on device -> fold into weights:
                        WuaT = A*W_u.T, WiaT = A*W_i.T, c = B@W_u.T+B@W_i.T+b_i
  pass2 (host-transposed featT, j-major cols: c = 128*j + g per supertile):
      zT = WuaT.T@featT + FL[seg]   (PE; FL added via periodic indicator matmul)
      t = tanh(0.5*zT + 0.5*c)      (ACT; sigmoid = .5+.5*tanh, const folds out
                                     of softmax so e' = (0.5*w_e)@t)
      e'-pack: per 128-node j-block, matmul(lhsT=th_block, rhs=we) -> one PSUM
               column => e' lands packed [128seg x 64j]; exp on packed (cheap)
      ex broadcast: PE-transpose -> row [1,8192] -> gpsimd partition_broadcast
      wft = ft*ex_rep (DVE 2x); segment sums via 6-level pairwise add tree
      (j-major makes tree levels contiguous)  -> sexT [f, g]
  host: rst = A*(sex/denom)+B ; position_rst = A*P + B*spw
"""

import numpy as np

N_NODES = 1048576
N_SEG = 16384
SEG = 64
D = 128
EPS = 1e-5
NCORES = 8

_CACHE = {}
LAST_RESULT = None  # BassKernelResults of the most recent kernel() call


def _build_program(n_cores, S):
    """Build + compile the per-core program. S = nodes per shard."""
    import concourse.bass as bass
    import concourse.tile as tile
    from concourse import bacc, mybir

    NSEGS = S // SEG          # segments per shard
    NSUP = S // 8192          # supertiles (128 segs each)
    assert S % 8192 == 0

    nc = bacc.Bacc(
        "TRN2",
        target_bir_lowering=False,
        debug=False,
        enable_asserts=False,
        num_devices=n_cores,
    )
    dtf = mybir.dt.float32
    dth = mybir.dt.bfloat16
    F32 = mybir.ActivationFunctionType

    # featN2: p-major native layout, partition p col (b*32 + j)*D + d holds
    # feat[4096*b + 128*j + p, d] -- contiguous per-partition DMA runs
    featN2 = nc.dram_tensor("featN2", [D, S], dth, kind="ExternalInput").ap()
    chkb = nc.dram_tensor("chkb", [D, D], dth, kind="ExternalInput").ap()
    featT = nc.dram_tensor("featT", [D, S], dth, kind="ExternalInput").ap()
    lastT = nc.dram_tensor("lastT", [D, NSEGS], dth, kind="ExternalInput").ap()
    pwm = nc.dram_tensor("pwm", [D, S // D], dth, kind="ExternalInput").ap()
    wut = nc.dram_tensor("wut", [D, D], dtf, kind="ExternalInput").ap()
    wit = nc.dram_tensor("wit", [D, D], dtf, kind="ExternalInput").ap()
    smalls = nc.dram_tensor("smalls", [D, 4], dtf, kind="ExternalInput").ap()
    ind128 = nc.dram_tensor("ind128", [D, 512], dth, kind="ExternalInput").ap()
    ident = nc.dram_tensor("ident", [D, D], dtf, kind="ExternalInput").ap()
    identh = nc.dram_tensor("identh", [D, D], dth, kind="ExternalInput").ap()

    sexT = nc.dram_tensor("sexT", [D, NSEGS], dtf, kind="ExternalOutput").ap()
    den = nc.dram_tensor("den", [D, NSUP], dtf, kind="ExternalOutput").ap()
    poutT = nc.dram_tensor("poutT", [D, NSEGS], dtf, kind="ExternalOutput").ap()
    statsout = nc.dram_tensor("statsout", [D, 2], dtf, kind="ExternalOutput").ap()

    AL = mybir.AluOpType

    with tile.TileContext(nc) as tc:
        from contextlib import ExitStack

        with ExitStack() as ctx:
            singles = ctx.enter_context(tc.tile_pool(name="singles", bufs=1))

            wut_sb = singles.tile([D, D], dtf)
            nc.sync.dma_start(wut_sb[:], wut)
            wit_sb = singles.tile([D, D], dtf)
            nc.sync.dma_start(wit_sb[:], wit)
            smalls_sb = singles.tile([D, 4], dtf)
            nc.sync.dma_start(smalls_sb[:], smalls)
            ind128_sb = singles.tile([D, 512], dth)
            nc.sync.dma_start(ind128_sb[:], ind128)
            ident_sb = singles.tile([D, D], dtf)
            nc.sync.dma_start(ident_sb[:], ident)
            identh_sb = singles.tile([D, D], dth)
            nc.sync.dma_start(identh_sb[:], identh)
            chk_sb = singles.tile([D, D], dth)
            nc.sync.dma_start(chk_sb[:], chkb)
            gamma_c = smalls_sb[:, 0:1]
            beta_c = smalls_sb[:, 1:2]
            bi_c = smalls_sb[:, 2:3]
            we_c = smalls_sb[:, 3:4]
            we_half = singles.tile([D, 1], dtf)
            nc.vector.tensor_scalar_mul(we_half[:], we_c, 0.5)
            we_bf = singles.tile([D, 1], dth)
            nc.vector.tensor_copy(we_bf[:], we_half[:])
            bi_half = singles.tile([D, 1], dtf)
            nc.vector.tensor_scalar_mul(bi_half[:], bi_c, 0.5)

            # psum accumulators for global stats live through pass1+stats
            with tc.tile_pool(name="psacc", bufs=1, space="PSUM") as psacc:
             psum_gram = psacc.tile([D, D], dtf)    # feat.T@feat; diag = sumsq
             cs_accum = singles.tile([D, 1], dtf)
             nc.vector.memset(cs_accum[:], 0.0)

             # ---------------- PASS 1 : native layout ----------------
             with tc.tile_pool(name="p1in", bufs=3) as p1in, \
                  tc.tile_pool(name="p1pw", bufs=2) as p1pw, \
                  tc.tile_pool(name="pblk", bufs=1) as pblk, \
                  tc.tile_pool(name="psP", bufs=2, space="PSUM") as psP:

                 # combo3 per supertile: col 3t = pw upper half of tile t,
                 # 3t+1 = pw lower half, 3t+2 = ones (per-tile colsum lane)
                 combo = []
                 for k in range(2):
                     t = pblk.tile([D, 192], dth, tag=f"combo{k}")
                     nc.vector.memset(t[:], 0.0)
                     ap_ones = t[:, :].rearrange("p (t three) -> p t three",
                                                 three=3)[:, :, 2:3]
                     nc.vector.memset(ap_ones, 1.0)
                     combo.append(t)

                 for s in range(NSUP):
                     cmb = combo[s % 2]
                     pwm_st = p1pw.tile([D, SEG], dth)
                     nc.scalar.dma_start(pwm_st[:], pwm[:, SEG * s:SEG * (s + 1)])
                     nc.vector.tensor_copy(
                         cmb[0:SEG, :].rearrange("p (t three) -> p t three",
                                                 three=3)[:, :, 0:1],
                         pwm_st[0:SEG, :],
                     )
                     nc.vector.tensor_copy(
                         cmb[SEG:D, :].rearrange("p (t three) -> p t three",
                                                 three=3)[:, :, 1:2],
                         pwm_st[SEG:D, :],
                     )

                     psum_PT = psP.tile([D, 192], dtf)
                     for b in range(2):  # big tiles: 4096 nodes each
                         bi = 2 * s + b
                         ftn = p1in.tile([D, 32, D], dth)
                         nc.sync.dma_start(
                             ftn[:],
                             featN2[:, 4096 * bi:4096 * (bi + 1)].rearrange(
                                 "p (j d) -> p j d", d=D
                             ),
                         )
                         first = (s == 0 and b == 0)
                         last = (s == NSUP - 1 and b == 1)
                         for j in range(32):  # 128-node tiles; ftn slice is lhsT
                             t_sup = 32 * b + j
                             lhs = ftn[:, j, :]
                             nc.tensor.matmul(
                                 psum_gram[:], lhs, lhs,
                                 start=(first and j == 0),
                                 stop=(last and j == 31),
                             )
                             mm = nc.tensor.matmul(
                                 psum_PT[:, 3 * t_sup:3 * t_sup + 3],
                                 lhs, cmb[:, 3 * t_sup:3 * t_sup + 3],
                                 start=True, stop=True,
                             )
                             mm.ins.ldweights = False
                     PT_sb = p1pw.tile([D, 192], dtf, tag="PT_sb")
                     nc.vector.tensor_copy(PT_sb[:], psum_PT[:])
                     # compact P pairs (cols 3t,3t+1) then one contiguous DMA
                     P_pack = p1pw.tile([D, D], dtf, tag="P_pack")
                     nc.vector.tensor_copy(
                         P_pack[:].rearrange("p (t two) -> p t two", two=2),
                         PT_sb[:].rearrange("p (t three) -> p t three",
                                            three=3)[:, :, 0:2],
                     )
                     nc.scalar.dma_start(poutT[:, D * s:D * (s + 1)], P_pack[:])
                     # colsum lanes (cols 3t+2) -> accumulate
                     csred = p1pw.tile([D, 1], dtf, tag="csred")
                     nc.vector.tensor_reduce(
                         csred[:],
                         PT_sb[:].rearrange("p (t three) -> p t three",
                                            three=3)[:, :, 2:3],
                         axis=mybir.AxisListType.XY, op=AL.add,
                     )
                     nc.vector.tensor_add(cs_accum[:], cs_accum[:], csred[:])

             # ---------------- STATS: allreduce + fold ----------------
             with tc.tile_pool(name="dram", bufs=1, space="DRAM") as dram, \
                  tc.tile_pool(name="stat", bufs=1) as stat, \
                  tc.tile_pool(name="psstat", bufs=1, space="PSUM") as psstat:
                 stats_col = stat.tile([D, 2], dtf)
                 nc.vector.tensor_copy(stats_col[:, 0:1], cs_accum[:])
                 # sumsq = diag(gram): mask with identity and row-reduce
                 gram_sb = stat.tile([D, D], dtf)
                 nc.vector.tensor_copy(gram_sb[:], psum_gram[:])
                 gmask = stat.tile([D, D], dtf)
                 nc.vector.tensor_mul(gmask[:], gram_sb[:], ident_sb[:])
                 nc.vector.tensor_reduce(
                     stats_col[:, 1:2], gmask[:],
                     axis=mybir.AxisListType.X, op=AL.add,
                 )
                 cc_in = dram.tile([D, 2], dtf)
                 cc_out = dram.tile([D, 2], dtf)
                 nc.sync.dma_start(cc_in[:], stats_col[:])
                 nc.gpsimd.collective_compute(
                     "AllReduce",
                     AL.add,
                     replica_groups=[list(range(n_cores))],
                     ins=[cc_in[:].opt()],
                     outs=[cc_out[:].opt()],
                 )
                 gstats = stat.tile([D, 2], dtf)
                 nc.sync.dma_start(gstats[:], cc_out[:])
                 nc.sync.dma_start(statsout, gstats[:])

                 n_tot = float(n_cores * S)
                 mean_c = stat.tile([D, 1], dtf)
                 nc.vector.tensor_scalar_mul(mean_c[:], gstats[:, 0:1], 1.0 / n_tot)
                 ex2_c = stat.tile([D, 1], dtf)
                 nc.vector.tensor_scalar_mul(ex2_c[:], gstats[:, 1:2], 1.0 / n_tot)
                 m2 = stat.tile([D, 1], dtf)
                 nc.vector.tensor_mul(m2[:], mean_c[:], mean_c[:])
                 var_c = stat.tile([D, 1], dtf)
                 nc.vector.tensor_sub(var_c[:], ex2_c[:], m2[:])
                 eps_t = stat.tile([D, 1], dtf)
                 nc.vector.memset(eps_t[:], EPS)
                 sd_c = stat.tile([D, 1], dtf)
                 nc.scalar.activation(sd_c[:], var_c[:], F32.Sqrt, bias=eps_t[:], scale=1.0)
                 rstd_c = stat.tile([D, 1], dtf)
                 nc.vector.reciprocal(rstd_c[:], sd_c[:])
                 A_c = stat.tile([D, 1], dtf)
                 nc.vector.tensor_mul(A_c[:], rstd_c[:], gamma_c)
                 mA = stat.tile([D, 1], dtf)
                 nc.vector.tensor_mul(mA[:], mean_c[:], A_c[:])
                 B_c = stat.tile([D, 1], dtf)
                 nc.vector.tensor_sub(B_c[:], beta_c, mA[:])

                 wuat = singles.tile([D, D], dth)
                 nc.vector.tensor_scalar_mul(wuat[:], wut_sb[:], A_c[:])
                 wiat = singles.tile([D, D], dth)
                 nc.vector.tensor_scalar_mul(wiat[:], wit_sb[:], A_c[:])
                 wsum = stat.tile([D, D], dtf)
                 nc.vector.tensor_add(wsum[:], wut_sb[:], wit_sb[:])
                 ps_c = psstat.tile([D, 1], dtf)
                 nc.tensor.matmul(ps_c[:], wsum[:], B_c[:], start=True, stop=True)
                 c_half = singles.tile([D, 1], dtf)
                 nc.vector.scalar_tensor_tensor(
                     c_half[:], ps_c[:], 0.5, bi_half[:], AL.mult, AL.add
                 )

            # ---------------- PASS 2 : transposed layout (j-major) ----------
            # fl pre-pass: FL[g,h] for all segs (frees psum banks for main loop)
            fl_all = singles.tile([D, NSUP, D], dth)
            with tc.tile_pool(name="flpre", bufs=2) as flprep, \
                 tc.tile_pool(name="psflp", bufs=2, space="PSUM") as psflp:
                for s in range(NSUP):
                    lt = flprep.tile([D, D], dth)
                    nc.scalar.dma_start(lt[:], lastT[:, D * s:D * (s + 1)])
                    psum_fl = psflp.tile([D, D], dtf)
                    nc.tensor.matmul(psum_fl[:], lt[:], wiat[:], start=True, stop=True)
                    nc.vector.tensor_copy(fl_all[:, s, :], psum_fl[:])

            with tc.tile_pool(name="ft", bufs=3) as ftp, \
                 tc.tile_pool(name="ftn2", bufs=4) as ftn2p, \
                 tc.tile_pool(name="th", bufs=3) as thp, \
                 tc.tile_pool(name="expk", bufs=2) as expkp, \
                 tc.tile_pool(name="ext", bufs=2) as extp, \
                 tc.tile_pool(name="exn", bufs=2) as exnp, \
                 tc.tile_pool(name="sout", bufs=2) as soutp, \
                 tc.tile_pool(name="denall", bufs=1) as denallp, \
                 tc.tile_pool(name="psz", bufs=2, space="PSUM") as psz, \
                 tc.tile_pool(name="pse", bufs=1, space="PSUM") as pse, \
                 tc.tile_pool(name="ptr", bufs=1, space="PSUM") as ptrp, \
                 tc.tile_pool(name="psx", bufs=2, space="PSUM") as psxp:

                den_all = denallp.tile([D, NSUP], dtf)

                for s in range(NSUP):
                    ft = ftp.tile([D, 8192], dth)
                    nc.sync.dma_start(ft[:], featT[:, 8192 * s:8192 * (s + 1)])
                    # native-layout tiles for the segment-sum matmuls
                    ftn2 = []
                    for b in range(2):
                        bi = 2 * s + b
                        t = ftn2p.tile([D, 32, D], dth)
                        nc.sync.dma_start(
                            t[:],
                            featN2[:, 4096 * bi:4096 * (bi + 1)].rearrange(
                                "p (j d) -> p j d", d=D
                            ),
                        )
                        ftn2.append(t)

                    psum_e = pse.tile([D, SEG], dtf)   # e' packed [g, j]
                    # phase A: scores; 8 psum tiles of 1024 cols (8 j-blocks)
                    for t in range(8):
                        psum_z = psz.tile([D, 1024], dtf)
                        for m in range(2):
                            mm = nc.tensor.matmul(
                                psum_z[:, 512 * m:512 * (m + 1)],
                                wuat[:],
                                ft[:, 1024 * t + 512 * m:1024 * t + 512 * (m + 1)],
                                start=True, stop=False,
                            )
                            if m == 1:
                                mm.ins.ldweights = False
                        for m in range(2):
                            mm = nc.tensor.matmul(
                                psum_z[:, 512 * m:512 * (m + 1)],
                                fl_all[:, s, :],
                                ind128_sb[:],
                                start=False, stop=True,
                            )
                            if m == 1:
                                mm.ins.ldweights = False
                        th_t = thp.tile([D, 1024], dth)
                        nc.scalar.activation(
                            th_t[:], psum_z[:], F32.Tanh, bias=c_half[:], scale=0.5
                        )
                        # e'-pack: one PSUM column per 128-node j-block
                        for b in range(8):
                            jj = 8 * t + b
                            nc.tensor.matmul(
                                psum_e[:, jj:jj + 1],
                                th_t[:, D * b:D * (b + 1)],
                                we_bf[:],
                                start=True, stop=True,
                            )
                    # exp on packed + denominators
                    ex_pk = expkp.tile([D, SEG], dth)
                    nc.scalar.activation(ex_pk[:], psum_e[:], F32.Exp)
                    nc.vector.tensor_reduce(
                        den_all[:, s:s + 1], ex_pk[:],
                        axis=mybir.AxisListType.X, op=AL.add,
                    )
                    # exN construction: transpose ex_pk -> [j, g], double it
                    # vertically, mask with the checkerboard. exN[p, 2t+h] =
                    # ex of node p in 128-node tile t (0 on the other seg).
                    ptr_t = ptrp.tile([SEG, D], dth)
                    nc.tensor.transpose(ptr_t[:], ex_pk[:], identh_sb[:])
                    exT = extp.tile([SEG, D], dth)
                    nc.vector.tensor_copy(exT[:], ptr_t[:])
                    exTT = exnp.tile([D, D], dth, tag="exTT")
                    nc.gpsimd.dma_start(exTT[0:SEG, :], exT[:])
                    nc.gpsimd.dma_start(exTT[SEG:D, :], exT[:])
                    exN = exnp.tile([D, D], dth, tag="exN")
                    nc.vector.tensor_mul(exN[:], exTT[:], chk_sb[:])

                    # segment sums on PE: contraction over the 128 nodes of
                    # each native tile, 2 output segments per tile
                    psum_sex = psxp.tile([D, D], dtf)
                    for t in range(SEG):
                        b, j = t // 32, t % 32
                        nc.tensor.matmul(
                            psum_sex[:, 2 * t:2 * t + 2],
                            ftn2[b][:, j, :],
                            exN[:, 2 * t:2 * t + 2],
                            start=True, stop=True,
                        )
                    sexG = soutp.tile([D, D], dtf)
                    nc.vector.tensor_copy(sexG[:], psum_sex[:])
                    nc.scalar.dma_start(sexT[:, D * s:D * (s + 1)], sexG[:])

                nc.scalar.dma_start(den, den_all[:])

    nc.compile()
    return nc


def _get_program(n_cores, S):
    key = (n_cores, S)
    if key not in _CACHE:
        _CACHE[key] = _build_program(n_cores, S)
    return _CACHE[key]


def _prep_core_inputs(feat_sh, pw_sh, W_u, W_i, b_i, w_e, gamma, beta):
    S = feat_sh.shape[0]
    NSUP = S // 8192
    import ml_dtypes
    f16 = ml_dtypes.bfloat16
    # j-major transposed layout: col (s, j, g) = node 8192*s + 64*g + j
    featT = np.ascontiguousarray(
        feat_sh.reshape(NSUP, 128, SEG, D).transpose(3, 0, 2, 1).reshape(D, S)
    ).astype(f16)
    lastT = np.ascontiguousarray(feat_sh[SEG - 1::SEG, :].T).astype(f16)
    pwm = np.ascontiguousarray(pw_sh.reshape(-1, D).T).astype(f16)
    # FL periodic indicator: ind128[g, c] = 1 iff c % 128 == g
    ind128 = np.tile(np.eye(D, dtype=np.float32), (1, 4)).astype(f16)
    smalls = np.stack([gamma, beta, b_i, w_e], axis=1).astype(np.float32)
    # p-major native layout: partition p, col (b*32+j)*D + d = feat[4096b+128j+p, d]
    featN2 = np.ascontiguousarray(
        feat_sh.reshape(-1, 32, D, D).transpose(2, 0, 1, 3).reshape(D, -1)
    ).astype(f16)
    # checkerboard mask: 1 iff p//64 == c%2
    chkb = np.fromfunction(
        lambda p, c: ((p // 64) == (c % 2)).astype(np.float32), (D, D)
    ).astype(f16)
    return {
        "featN2": featN2,
        "chkb": chkb,
        "featT": featT,
        "lastT": lastT,
        "pwm": pwm,
        "wut": np.ascontiguousarray(W_u.T),
        "wit": np.ascontiguousarray(W_i.T),
        "smalls": smalls,
        "ind128": ind128,
        "ident": np.eye(D, dtype=np.float32),
        "identh": np.eye(D, dtype=np.float32).astype(f16),
    }


def _finalize(results, n_cores, S, gamma, beta, pw):
    NSEGS = S // SEG
    spw_all = pw.astype(np.float64).reshape(-1, SEG).sum(1).astype(np.float32)
    st = results[0]["statsout"]            # [D, 2]
    n_tot = float(n_cores * S)
    mean = st[:, 0] / n_tot
    var = st[:, 1] / n_tot - mean * mean
    A = gamma / np.sqrt(var + EPS)
    B = beta - mean * A
    rst = np.empty((n_cores * NSEGS, D), dtype=np.float32)
    pos = np.empty((n_cores * NSEGS, D), dtype=np.float32)
    for c in range(n_cores):
        r = results[c]
        sex = r["sexT"].T                      # [NSEGS, D]
        denom = r["den"].T.reshape(-1)         # seg order
        p = r["poutT"].T                       # [NSEGS, D]
        sl = slice(c * NSEGS, (c + 1) * NSEGS)
        spw = spw_all[sl]
        rst[sl] = A * (sex / denom[:, None]) + B
        pos[sl] = A * p + B * spw[:, None]
    return rst, pos


def kernel(feat, position_weight, last_nodes, segment_ids, gamma, beta,
           W_u, W_i, b_i, w_e, num_segments):
    from concourse.bass_utils import run_bass_kernel_spmd

    feat = np.asarray(feat, dtype=np.float32)
    pw = np.asarray(position_weight, dtype=np.float32)
    gamma = np.asarray(gamma, dtype=np.float32)
    beta = np.asarray(beta, dtype=np.float32)
    W_u = np.asarray(W_u, dtype=np.float32)
    W_i = np.asarray(W_i, dtype=np.float32)
    b_i = np.asarray(b_i, dtype=np.float32)
    w_e = np.asarray(w_e, dtype=np.float32)

    n = feat.shape[0]
    assert n == N_NODES and feat.shape[1] == D
    S = n // NCORES

    nc = _get_program(NCORES, S)
    in_maps = []
    for c in range(NCORES):
        sl = slice(c * S, (c + 1) * S)
        in_maps.append(
            _prep_core_inputs(feat[sl], pw[sl], W_u, W_i, b_i, w_e, gamma, beta)
        )
    import os
    trace = bool(int(os.environ.get("ATTN_TRACE", "0")))
    res = run_bass_kernel_spmd(nc, in_maps, list(range(NCORES)), trace=trace)
    global LAST_RESULT
    LAST_RESULT = res
    rst, pos = _finalize(res.results, NCORES, S, gamma, beta, pw)
    return rst, pos
